# revision 36
# baseline (speedup 1.0000x reference)
# Trainium2 Bass kernel for nn_Network_515396076038 (nms_detection / OICR-style loss).
#
# v3 strategy (8 NeuronCores, data-parallel over the N=4096 proposals):
#   - Inputs stream in bf16 (host-cast): ~12.9 MB/core, PE runs bf16 matmuls
#     at 1 cycle/row. fc7 shards are host-packed to [128, KT, NS] so every
#     big DMA is 128 contiguous per-partition runs. Small DMAs are queued
#     after the first roi chunk so the PE starts ASAP.
#   - det head: frame-context subtract on DVE/GpSimd (idle during the GEMM),
#     one det GEMM instead of two.
#   - All post-GEMM elementwise stats run ROI-MAJOR ([128, NB, *] tiles, full
#     128 DVE lanes) after tiny PE transposes; per-class sums use matmuls,
#     the per-class argmax max uses transpose + free-dim reduce. Candidate
#     box AREAS ride the same sel-mask gather matmul (5th lhsT column) and
#     ship in the AllGather payload, shortening the post-collective chain.
#   - The r2 refine head GEMM + its log-softmax run AFTER the AllGather
#     trigger (t_roi stays resident in SBUF), filling collective latency.
#   - Log-softmax needs no max-subtraction (|scores| < ~4): lp = x - ln(sum exp x).
#   - One AllGather total. The final loss reduction is done on host from
#     per-core partial sums (the gather/unshard step).
import sys

for _p in ("/opt/trn_rl_repo",):
    if _p not in sys.path:
        sys.path.append(_p)

import ml_dtypes
import numpy as np

import concourse.bass as bass
import concourse.bass_isa as bass_isa
import concourse.mybir as mybir
import concourse.tile as tile
from concourse import bacc
from concourse.bass_utils import run_bass_kernel_spmd
from concourse.masks import make_identity

dt = mybir.dt
Alu = mybir.AluOpType
Act = mybir.ActivationFunctionType
AX = mybir.AxisListType

C = 20       # foreground classes
CR = C + 1   # refine head classes (background + C)
CW = C + CR  # stacked critical-path roi heads: cls | r1 = 41
PA = 48      # w_a padded cols (fp8 DoubleRow needs 16B-aligned k-stride)
PD = 48      # w_det padded cols: W(20) | -W(20) | pad(8)
PR = 32      # w_r2 padded cols


def _emit(nc, tc, aps, NS, F, n_cores):
    NB = NS // 128   # 4 roi blocks
    KT = F // 128    # 32 contraction slices
    KH = KT // 2     # roi chunk size
    KQ = KT // 4     # frm/ctx chunk size
    NP = NB * 2      # (block, supervision) pairs, index b*2+s
    group = [list(range(n_cores))]
    GW = 281  # AllGather row: vm[41] | boxes+areas[200] | z[20] | s1[20]

    (fc7, w_a, w_det, w_r2, b_a, b_r2, bxw, lab, out) = aps

    const = tc.alloc_tile_pool(name="const", bufs=1)
    st = tc.alloc_tile_pool(name="st", bufs=1)
    pst = tc.alloc_tile_pool(name="pst", bufs=2, space="PSUM")
    pss = tc.alloc_tile_pool(name="pss", bufs=2, space="PSUM")
    psa = tc.alloc_tile_pool(name="psa", bufs=1, space="PSUM")
    dp = tc.alloc_tile_pool(name="dp", bufs=1, space="DRAM")
    psc = tc.alloc_tile_pool(name="psc", bufs=1, space="PSUM")

    # ---------------- DMA issue order = arrival order ----------------------
    fc7_sb = st.tile([128, 3 * KT, NS], dt.float8e4)   # roi | frm | ctx
    w_a_sb = const.tile([128, KT, PA], dt.float8e4)
    nc.sync.dma_start(w_a_sb, w_a)
    nc.sync.dma_start(fc7_sb[:, 0:KT, :], fc7[:, 0:KT, :])
    w_det_sb = const.tile([128, KT, PD], dt.float8e4)
    nc.sync.dma_start(w_det_sb, w_det)
    b_a_sb = const.tile([CW, 1], dt.float32)
    nc.sync.dma_start(b_a_sb, b_a[:, None])
    b_r2_sb = const.tile([CR, 1], dt.float32)
    nc.sync.dma_start(b_r2_sb, b_r2[:, None])
    bxw_sb = st.tile([128, NB, 5], dt.float32)
    nc.sync.dma_start(bxw_sb, bxw)
    labrow_i = st.tile([1, C], dt.int32)
    nc.sync.dma_start(labrow_i, lab)
    nc.sync.dma_start(fc7_sb[:, KT:2 * KT, :], fc7[:, KT:2 * KT, :])
    nc.sync.dma_start(fc7_sb[:, 2 * KT:3 * KT, :], fc7[:, 2 * KT:3 * KT, :])
    w_r2_sb = const.tile([128, KT, PR], dt.float8e4)
    nc.sync.dma_start(w_r2_sb, w_r2)
    boxes_nat = bxw_sb[:, :, 0:4]
    isw_col = bxw_sb[:, :, 4:5]     # [128, NB, 1]

    # ---------------- constants ----------------
    ident = const.tile([128, 128], dt.float32)
    make_identity(nc, ident)
    ones_col = const.tile([128, 1], dt.float32)
    nc.vector.memset(ones_col, 1.0)
    ones_row = const.tile([1, 128], dt.float32)
    nc.vector.memset(ones_row, 1.0)
    ones_pp = const.tile([128, 128], dt.float32)
    nc.vector.memset(ones_pp, 1.0)
    iota_i = const.tile([128, CR], dt.int32)
    nc.gpsimd.iota(iota_i, pattern=[[1, CR]], base=0, channel_multiplier=0)
    iota_f = const.tile([128, CR], dt.float32)
    nc.vector.tensor_copy(iota_f, iota_i)
    iota_m1k = const.tile([128, C], dt.float32)
    nc.vector.tensor_scalar_add(iota_m1k, iota_f[:, :C], -1000.0)
    warmln = const.tile([1, 1], dt.float32)
    nc.scalar.activation(warmln, ones_col[0:1, :], Act.Ln)   # preload Ln table

    labrow_f = st.tile([1, C], dt.float32)
    nc.vector.tensor_copy(labrow_f, labrow_i)

    # roi areas + boxes|area pack for the sel gather (early, off critical path)
    ab_all = st.tile([128, NB, 1], dt.float32)
    tw = st.tile([128, NB, 1], dt.float32)
    nc.vector.tensor_tensor(ab_all, bxw_sb[:, :, 2:3], bxw_sb[:, :, 0:1], Alu.subtract)
    nc.vector.tensor_scalar_add(ab_all, ab_all, 1.0)
    nc.vector.tensor_tensor(tw, bxw_sb[:, :, 3:4], bxw_sb[:, :, 1:2], Alu.subtract)
    nc.vector.tensor_scalar_add(tw, tw, 1.0)
    nc.vector.tensor_mul(ab_all, ab_all, tw)
    bxa = st.tile([128, NB, 5], dt.float32)
    nc.vector.tensor_copy(bxa[:, :, 0:4], boxes_nat)
    nc.vector.tensor_copy(bxa[:, :, 4:5], ab_all)

    # (1-mask)*1e30 broadcast over the 8 core-rows (filler, local-only):
    # added to negative classes' query areas so their IoU becomes ~0
    big_row = st.tile([1, 2 * C], dt.float32)
    nc.vector.tensor_scalar(big_row[:, 0:C], labrow_f, 0.0, 1e30, Alu.is_lt, Alu.mult)
    nc.vector.tensor_copy(big_row[:, C:2 * C], big_row[:, 0:C])
    ps_m = pss.tile([128, 512], dt.float32, tag="mm")
    nc.tensor.matmul(ps_m[0:n_cores, 0:2 * C], ones_row[0:1, 0:n_cores], big_row,
                     start=True, stop=True)
    bigmask8 = st.tile([n_cores, 2 * C], dt.float32)
    nc.vector.tensor_copy(bigmask8, ps_m[0:n_cores, 0:2 * C])

    # ---------------- main GEMM (bf16) -------------------------------------
    # det = frm@W + ctx@(-W): accumulate both into one PSUM bank; [W|-W] is
    # host-packed so no vector subtract is needed.
    scoresA = psc.tile([128, NS], dt.float32)   # rows 0:CW = cls | r1
    scoresB = psc.tile([128, NS], dt.float32)   # rows 0:C  = det (frm - ctx)
    DR = mybir.MatmulPerfMode.DoubleRow
    for j in range(KT // 2):
        nc.tensor.matmul(scoresA[0:CW, :], w_a_sb[:, 2 * j:2 * j + 2, 0:CW],
                         fc7_sb[:, 2 * j:2 * j + 2, :],
                         start=(j == 0), stop=(j == KT // 2 - 1), perf_mode=DR)
    for h in range(2):
        for j in range(KQ):
            kk = h * KH + 2 * j
            nc.tensor.matmul(scoresB[0:C, :], w_det_sb[:, kk:kk + 2, 0:C],
                             fc7_sb[:, KT + kk:KT + kk + 2, :],
                             start=(kk == 0), stop=False, perf_mode=DR)
        for j in range(KQ):
            kk = h * KH + 2 * j
            nc.tensor.matmul(scoresB[0:C, :], w_det_sb[:, kk:kk + 2, C:2 * C],
                             fc7_sb[:, 2 * KT + kk:2 * KT + kk + 2, :],
                             start=False, stop=(kk == KT - 2), perf_mode=DR)

    # ---------------- roi-major stats (critical path to the AllGather) -----
    CD = CW + C  # 61 packed roi-major columns
    sA = st.tile([CW, NS], dt.float32)
    nc.scalar.activation(sA, scoresA[0:CW, :], Act.Identity, bias=b_a_sb)
    dE = st.tile([C, NS], dt.float32)
    nc.scalar.activation(dE, scoresB[0:C, :], Act.Exp)

    rme = st.tile([128, NB, CD], dt.float32)     # cls|r1|exp(det), roi-major
    for b in range(NB):
        bsl = bass.ts(b, 128)
        ptx = pst.tile([128, 128], dt.float32, tag="pt")
        nc.tensor.transpose(ptx[:, 0:CW], sA[:, bsl], ident[0:CW, 0:CW])
        nc.tensor.transpose(ptx[:, CW:CD], dE[:, bsl], ident[0:C, 0:C])
        nc.vector.tensor_copy(rme[:, b, :], ptx[:, 0:CD])
    rm = rme[:, :, 0:CW]
    ed = rme[:, :, CW:CD]

    e = st.tile([128, NB, CW], dt.float32)       # exp(cls)|exp(r1)
    nc.scalar.activation(e, rm, Act.Exp)
    ec = e[:, :, 0:C]
    er = e[:, :, C:CW]
    scls = st.tile([128, NB, 1], dt.float32)
    nc.vector.reduce_sum(scls, ec, axis=AX.X)
    sr1 = st.tile([128, NB, 1], dt.float32)
    nc.vector.reduce_sum(sr1, er, axis=AX.X)
    rb1 = st.tile([128, NB, 1], dt.float32)
    nc.vector.reciprocal(rb1, scls)
    nc.vector.tensor_mul(rb1, rb1, isw_col)
    rb2 = st.tile([128, NB, 1], dt.float32)
    nc.vector.reciprocal(rb2, sr1)
    nc.vector.tensor_mul(rb2, rb2, isw_col)

    pq = st.tile([128, NB, CW], dt.float32)      # p1(20) | q2(21)
    nc.vector.tensor_mul(pq[:, :, 0:C], ec, ed)
    nc.vector.tensor_tensor(
        pq[:, :, 0:C], pq[:, :, 0:C], rb1.to_broadcast([128, NB, C]), Alu.mult)
    nc.vector.tensor_tensor(
        pq[:, :, C:CW], er, rb2.to_broadcast([128, NB, CR]), Alu.mult)

    # z / s1 partial sums: [1, 40] row via ones-matmul over rois
    zsp = st.tile([128, NB, 2 * C], dt.float32)  # exp(det) | cls*exp(det)
    nc.vector.tensor_copy(zsp[:, :, 0:C], ed)
    nc.vector.tensor_mul(zsp[:, :, C:2 * C], rm[:, :, 0:C], ed)
    ps_z = pss.tile([128, 512], dt.float32, tag="mm")
    for b in range(NB):
        nc.tensor.matmul(ps_z[0:1, 0:2 * C], ones_col, zsp[:, b, :],
                         start=(b == 0), stop=(b == NB - 1))
    zrow = st.tile([1, 2 * C], dt.float32)
    nc.vector.tensor_copy(zrow, ps_z[0:1, 0:2 * C])

    # per-class max over rois: free-dim max over blocks, then a GpSimd
    # cross-partition all-reduce (result is already broadcast to all rows)
    pmax = st.tile([128, CW], dt.float32)
    nc.vector.tensor_reduce(pmax, pq.rearrange("p b c -> p c b"),
                            axis=AX.X, op=Alu.max)
    vmP = st.tile([128, CW], dt.float32)
    nc.gpsimd.partition_all_reduce(
        vmP, pmax, channels=128, reduce_op=bass_isa.ReduceOp.max)

    sel = st.tile([128, NB, CW], dt.float32)
    nc.vector.tensor_tensor(
        sel, pq, vmP[:, None, :].to_broadcast([128, NB, CW]), Alu.is_equal)
    psq = psa.tile([128, 512], dt.float32, tag="acc")
    for b in range(NB):
        nc.tensor.matmul(psq[0:5, 0:CW], bxa[:, b, :], sel[:, b, :],
                         start=(b == 0), stop=(b == NB - 1))
    bc_sb = st.tile([5, 2 * C], dt.float32)      # winner boxes+areas, s-paired
    nc.vector.tensor_copy(bc_sb[:, 0:C], psq[0:5, 0:C])
    nc.vector.tensor_copy(bc_sb[:, C:2 * C], psq[0:5, CR:CW])   # skip bg col
    vmz = st.tile([1, 81], dt.float32)           # vm row | z | s1
    nc.vector.tensor_copy(vmz[:, 0:CW], vmP[0:1, :])
    nc.vector.tensor_copy(vmz[:, CW:81], zrow)

    # ---------------- G1: AllGather of all cross-core state ----------------
    # layout: vm[0:41] | z,s1[41:81] | boxes+areas[81:281]
    g1_in = dp.tile([GW], dt.float32)
    g1_out = dp.tile([n_cores * GW], dt.float32)
    nc.scalar.dma_start(g1_in[0:81], vmz)
    nc.sync.dma_start(g1_in[81:281], bc_sb)
    nc.gpsimd.collective_compute(
        "AllGather", Alu.bypass, replica_groups=group,
        ins=[g1_in.opt()], outs=[g1_out.opt()],
    )

    # ---- collective-latency filler: r2 GEMM + both heads' log-softmax -----
    # scoresR reuses psq's PSUM buffer (psa pool): the WAR dependency keeps
    # the PE from hoisting the r2 GEMM in front of the argmax/gather chain.
    scoresR = psa.tile([128, 512], dt.float32, tag="acc")
    for j in range(KT // 2):
        nc.tensor.matmul(scoresR[0:CR, :], w_r2_sb[:, 2 * j:2 * j + 2, 0:CR],
                         fc7_sb[:, 2 * j:2 * j + 2, :],
                         start=(j == 0), stop=(j == KT // 2 - 1), perf_mode=DR)
    r2c = st.tile([CR, NS], dt.float32)
    nc.scalar.activation(r2c, scoresR[0:CR, :], Act.Identity, bias=b_r2_sb)
    r2m = st.tile([128, NB, CR], dt.float32)
    for b in range(NB):
        bsl = bass.ts(b, 128)
        ptr = pst.tile([128, 128], dt.float32, tag="pt")
        nc.tensor.transpose(ptr[:, 0:CR], r2c[:, bsl], ident[0:CR, 0:CR])
        nc.vector.tensor_copy(r2m[:, b, :], ptr[:, 0:CR])
    er2 = st.tile([128, NB, CR], dt.float32)
    nc.scalar.activation(er2, r2m, Act.Exp)
    sr2 = st.tile([128, NB, 1], dt.float32)
    nc.vector.reduce_sum(sr2, er2, axis=AX.X)

    # log-probs for both supervisions: x - ln(sum exp x); pairs (b, s)
    xs = st.tile([128, NB, 2, CR], dt.float32)
    ln1 = st.tile([128, NB, 1], dt.float32)
    nc.scalar.activation(ln1, sr1, Act.Ln)
    nc.vector.tensor_tensor(
        xs[:, :, 0, :], rm[:, :, C:CW],
        ln1.to_broadcast([128, NB, CR]), Alu.subtract)
    ln2 = st.tile([128, NB, 1], dt.float32)
    nc.scalar.activation(ln2, sr2, Act.Ln)
    nc.vector.tensor_tensor(
        xs[:, :, 1, :], r2m, ln2.to_broadcast([128, NB, CR]), Alu.subtract)

    # ---------------- G1 readback + cross-core combine ----------------
    g_sb = st.tile([n_cores, GW], dt.float32)
    nc.sync.dma_start(g_sb, g1_out.rearrange("(r w) -> r w", r=n_cores))
    vmx = st.tile([n_cores, CW], dt.float32)
    nc.gpsimd.partition_all_reduce(
        vmx, g_sb[:, 0:CW], channels=n_cores, reduce_op=bass_isa.ReduceOp.max
    )
    selc = st.tile([n_cores, CW], dt.float32)
    nc.vector.tensor_tensor(selc, g_sb[:, 0:CW], vmx, Alu.is_equal)
    masked = st.tile([n_cores, 240], dt.float32)
    mview = masked[:, 0:200].rearrange("p (co s c) -> p co s c", co=5, s=2)
    gview = g_sb[:, 81:281].rearrange("p (co s c) -> p co s c", co=5, s=2)
    nc.vector.tensor_tensor(
        mview[:, :, 0, :], gview[:, :, 0, :],
        selc[:, None, 0:C].to_broadcast([n_cores, 5, C]), Alu.mult,
    )
    nc.vector.tensor_tensor(
        mview[:, :, 1, :], gview[:, :, 1, :],
        selc[:, None, CR:CW].to_broadcast([n_cores, 5, C]), Alu.mult,
    )
    nc.vector.tensor_tensor(masked[:, 160:200], masked[:, 160:200],
                            bigmask8, Alu.add)
    nc.vector.tensor_copy(masked[:, 200:240], g_sb[:, CW:81])
    # one matmul: ones[8,128].T @ masked -> core-sums broadcast to all rows
    ps_q = pss.tile([128, 512], dt.float32, tag="mm")
    nc.tensor.matmul(ps_q[:, 0:240], ones_pp[0:n_cores, :], masked,
                     start=True, stop=True)
    QA = st.tile([128, 240], dt.float32)   # x1|y1|x2|y2|area pairs, z|s1 tail
    nc.vector.tensor_copy(QA, ps_q[:, 0:240])

    # ---------------- batched IoU / assignment / loss ----------------------
    W2 = 2 * C
    def qb(lo):   # query coord block [128, 1, 2C] -> [128, NB, 2C]
        return QA[:, None, lo:lo + W2].to_broadcast([128, NB, W2])
    def bb(i):    # per-block box coord [128, NB, 1] -> [128, NB, 2C]
        return boxes_nat[:, :, i:i + 1].to_broadcast([128, NB, W2])

    aqab = st.tile([128, NB, W2], dt.float32)    # area_q + area_b
    nc.vector.tensor_tensor(aqab, qb(160), ab_all.to_broadcast([128, NB, W2]), Alu.add)
    xi1 = st.tile([128, NB, W2], dt.float32)
    nc.vector.tensor_tensor(xi1, qb(0), bb(0), Alu.max)
    yi1 = st.tile([128, NB, W2], dt.float32)
    nc.vector.tensor_tensor(yi1, qb(40), bb(1), Alu.max)
    xi2 = st.tile([128, NB, W2], dt.float32)
    nc.vector.tensor_tensor(xi2, qb(80), bb(2), Alu.min)
    yi2 = st.tile([128, NB, W2], dt.float32)
    nc.vector.tensor_tensor(yi2, qb(120), bb(3), Alu.min)
    nc.vector.tensor_tensor(xi2, xi2, xi1, Alu.subtract)
    nc.vector.tensor_scalar(xi2, xi2, 1.0, 0.0, Alu.add, Alu.max)   # iw
    nc.vector.tensor_tensor(yi2, yi2, yi1, Alu.subtract)
    nc.vector.tensor_scalar(yi2, yi2, 1.0, 0.0, Alu.add, Alu.max)   # ih
    inter = st.tile([128, NB, W2], dt.float32)
    nc.vector.tensor_mul(inter, xi2, yi2)
    un = st.tile([128, NB, W2], dt.float32)
    nc.vector.tensor_tensor(un, aqab, inter, Alu.subtract)
    nc.vector.reciprocal(un, un)
    ov = st.tile([128, NB, W2], dt.float32)
    nc.vector.tensor_mul(ov, inter, un)

    ovp = ov.rearrange("p b (s c) -> p (b s) c", s=2)   # [128, NP, C]
    mo = st.tile([128, NP, 1], dt.float32)
    nc.vector.reduce_max(mo, ovp, axis=AX.X)
    meq = st.tile([128, NP, C], dt.float32)
    nc.vector.tensor_tensor(
        meq, ovp, mo.to_broadcast([128, NP, C]), Alu.is_equal)
    nc.vector.tensor_tensor(
        meq, meq, iota_m1k[:, None, :].to_broadcast([128, NP, C]), Alu.mult)
    gt = st.tile([128, NP, 1], dt.float32)
    nc.vector.tensor_reduce(gt, meq, axis=AX.X, op=Alu.min)
    nc.vector.tensor_scalar_add(gt, gt, 1001.0)          # argmax + 1

    fg = st.tile([128, NP, 1], dt.float32)
    nc.vector.tensor_scalar(fg, mo, 0.5, None, Alu.is_gt)
    keep = st.tile([128, NP, 1], dt.float32)
    nc.vector.tensor_scalar(keep, mo, 0.1, None, Alu.is_ge)
    col = st.tile([128, NP, 1], dt.float32)
    nc.vector.tensor_mul(col, gt, fg)                    # fg ? argmax+1 : 0
    oh = st.tile([128, NP, CR], dt.float32)
    nc.vector.tensor_tensor(
        oh, iota_f[:, None, :].to_broadcast([128, NP, CR]),
        col.to_broadcast([128, NP, CR]), Alu.is_equal)
    nc.vector.tensor_mul(oh, oh, xs.rearrange("p b s c -> p (b s) c"))
    lpsel = st.tile([128, NP, 1], dt.float32)
    nc.vector.reduce_sum(lpsel, oh, axis=AX.X)

    stats = st.tile([128, 16], dt.float32)               # wl[8] | keep[8]
    wv = stats[:, 0:NP].rearrange("p (b s) -> p b s", s=2)
    kv = keep.rearrange("p (b s) o -> p b (s o)", s=2)   # [128, NB, 2]
    nc.vector.tensor_tensor(wv, kv, isw_col.to_broadcast([128, NB, 2]), Alu.mult)
    nc.vector.tensor_mul(
        stats[:, 0:NP], stats[:, 0:NP],
        lpsel.rearrange("p n o -> p (n o)"))
    nc.vector.tensor_copy(stats[:, NP:2 * NP], keep.rearrange("p n o -> p (n o)"))
    ps_l = psa.tile([128, 512], dt.float32, tag="acc")
    nc.tensor.matmul(ps_l[0:16, 0:1], stats, ones_col, start=True, stop=True)
    lsum = st.tile([16, 1], dt.float32)
    nc.vector.tensor_copy(lsum, ps_l[0:16, 0:1])

    # hinge is finished on host from the global z|s1 sums (identical on
    # every core after the AllGather)
    nc.scalar.dma_start(out[16:56], QA[0:1, 200:240])
    nc.sync.dma_start(out[0:16], lsum[:, 0])

    for pool in (psc, dp, psa, pss, pst, st, const):
        pool.release()


def build_program(NS=512, F=4096, n_cores=8):
    nc = bacc.Bacc(
        "TRN2", target_bir_lowering=False, debug=False, num_devices=n_cores
    )
    KT = F // 128
    fc7 = nc.dram_tensor("fc7", [128, 3 * KT, NS], dt.float8e4, kind="ExternalInput").ap()
    w_a = nc.dram_tensor("w_a", [128, KT, PA], dt.float8e4, kind="ExternalInput").ap()
    w_det = nc.dram_tensor("w_det", [128, KT, PD], dt.float8e4, kind="ExternalInput").ap()
    w_r2 = nc.dram_tensor("w_r2", [128, KT, PR], dt.float8e4, kind="ExternalInput").ap()
    b_a = nc.dram_tensor("b_a", [CW], dt.float32, kind="ExternalInput").ap()
    b_r2 = nc.dram_tensor("b_r2", [CR], dt.float32, kind="ExternalInput").ap()
    bxw = nc.dram_tensor("bxw", [128, NS // 128, 5], dt.float32, kind="ExternalInput").ap()
    lab = nc.dram_tensor("lab", [1, C], dt.int32, kind="ExternalInput").ap()
    out = nc.dram_tensor("out", [56], dt.float32, kind="ExternalOutput").ap()
    aps = (fc7, w_a, w_det, w_r2, b_a, b_r2, bxw, lab, out)
    with tile.TileContext(nc) as tc:
        _emit(nc, tc, aps, NS, F, n_cores)
    nc.compile()
    return nc


def _pack_fc7(a_t_bf16, sl, F):
    # [F, NS] bf16 slice -> [128, KT, NS] with contiguous per-partition runs
    return np.ascontiguousarray(
        a_t_bf16[:, sl].reshape(F // 128, 128, -1).transpose(1, 0, 2))


def _pack_w(w, pad_cols):
    F, cols = w.shape
    wp = np.zeros((F, pad_cols), np.float32)
    wp[:, 0:cols] = w
    return np.ascontiguousarray(
        wp.astype(ml_dtypes.float8_e4m3fn).reshape(F // 128, 128, pad_cols)
        .transpose(1, 0, 2))


def make_in_maps(inputs, NS, n_cores):
    f32 = np.float32
    bf16 = ml_dtypes.bfloat16
    fp8 = ml_dtypes.float8_e4m3fn
    w_a = _pack_w(np.concatenate(
        [np.asarray(inputs["W_cls"], f32), np.asarray(inputs["W_r1"], f32)], axis=1), PA)
    wd = np.asarray(inputs["W_det"], f32)
    w_det = _pack_w(np.concatenate([wd, -wd], axis=1), PD)
    w_r2 = _pack_w(np.asarray(inputs["W_r2"], f32), PR)
    b_a = np.ascontiguousarray(np.concatenate(
        [np.asarray(inputs["b_cls"]), np.asarray(inputs["b_r1"])]), f32)
    b_r2 = np.ascontiguousarray(np.asarray(inputs["b_r2"]), f32)
    boxes = np.asarray(inputs["ss_boxes"], f32)[:, 1:5]
    iswf = np.asarray(inputs["IS_weight"], f32)[:, 0]
    lab = np.ascontiguousarray(np.asarray(inputs["image_level_label"]), np.int32)
    roi = np.asarray(inputs["fc7_roi"], f32).T.astype(fp8)
    frm = np.asarray(inputs["fc7_frame"], f32).T.astype(fp8)
    ctxm = np.asarray(inputs["fc7_context"], f32).T.astype(fp8)
    F = roi.shape[0]
    NB = NS // 128

    in_maps = []
    for c in range(n_cores):
        sl = slice(c * NS, (c + 1) * NS)
        bsh = boxes[sl].reshape(NB, 128, 4).transpose(1, 0, 2)
        ish = iswf[sl].reshape(NB, 128).T[:, :, None]
        bxw = np.ascontiguousarray(np.concatenate([bsh, ish], axis=2), f32)
        in_maps.append({
            "fc7": np.ascontiguousarray(np.concatenate(
                [_pack_fc7(roi, sl, F), _pack_fc7(frm, sl, F),
                 _pack_fc7(ctxm, sl, F)], axis=1)),
            "w_a": w_a, "w_det": w_det, "w_r2": w_r2,
            "b_a": b_a, "b_r2": b_r2,
            "bxw": bxw, "lab": lab,
        })
    return in_maps


_PROG_CACHE = {}


def _get_prog(NS, F, n_cores):
    key = (NS, F, n_cores)
    if key not in _PROG_CACHE:
        _PROG_CACHE[key] = build_program(NS, F, n_cores)
    return _PROG_CACHE[key]


def finish(results, lab, n_cores=8):
    # host-side gather/unshard: combine the per-core partial sums
    parts = np.stack([np.asarray(results[i]["out"], np.float64).reshape(56)
                      for i in range(n_cores)])
    wl = parts[:, 0:8].sum(axis=0)      # per (b, s=idx%2) weighted log-probs
    kp = parts[:, 8:16].sum(axis=0)     # per (b, s) keep counts
    rl1 = -wl[0::2].sum() / kp[0::2].sum()
    rl2 = -wl[1::2].sum() / kp[1::2].sum()
    z = parts[0, 16:36]
    s1 = parts[0, 36:56]
    h = np.maximum(0.0, 1.0 - lab * (s1 / z)).sum()
    return np.float32(h / C + 0.1 * rl1 + 0.1 * rl2)


def kernel(**inputs):
    n_cores = 8
    N, F = inputs["fc7_roi"].shape
    NS = N // n_cores
    prog = _get_prog(NS, F, n_cores)
    in_maps = make_in_maps(inputs, NS, n_cores)
    res = run_bass_kernel_spmd(prog, in_maps, list(range(n_cores))).results
    lab = np.asarray(inputs["image_level_label"], np.float64)[0]
    return finish(res, lab, n_cores)


# revision 38
# speedup vs baseline: 1.0188x; 1.0188x over previous
# Trainium2 Bass kernel for nn_Network_515396076038 (nms_detection / OICR-style loss).
#
# v3 strategy (8 NeuronCores, data-parallel over the N=4096 proposals):
#   - Inputs stream in bf16 (host-cast): ~12.9 MB/core, PE runs bf16 matmuls
#     at 1 cycle/row. fc7 shards are host-packed to [128, KT, NS] so every
#     big DMA is 128 contiguous per-partition runs. Small DMAs are queued
#     after the first roi chunk so the PE starts ASAP.
#   - det head: frame-context subtract on DVE/GpSimd (idle during the GEMM),
#     one det GEMM instead of two.
#   - All post-GEMM elementwise stats run ROI-MAJOR ([128, NB, *] tiles, full
#     128 DVE lanes) after tiny PE transposes; per-class sums use matmuls,
#     the per-class argmax max uses transpose + free-dim reduce. Candidate
#     box AREAS ride the same sel-mask gather matmul (5th lhsT column) and
#     ship in the AllGather payload, shortening the post-collective chain.
#   - The r2 refine head GEMM + its log-softmax run AFTER the AllGather
#     trigger (t_roi stays resident in SBUF), filling collective latency.
#   - Log-softmax needs no max-subtraction (|scores| < ~4): lp = x - ln(sum exp x).
#   - One AllGather total. The final loss reduction is done on host from
#     per-core partial sums (the gather/unshard step).
import sys

for _p in ("/opt/trn_rl_repo",):
    if _p not in sys.path:
        sys.path.append(_p)

import ml_dtypes
import numpy as np

import concourse.bass as bass
import concourse.bass_isa as bass_isa
import concourse.mybir as mybir
import concourse.tile as tile
from concourse import bacc
from concourse.bass_utils import run_bass_kernel_spmd
from concourse.masks import make_identity

dt = mybir.dt
Alu = mybir.AluOpType
Act = mybir.ActivationFunctionType
AX = mybir.AxisListType

C = 20       # foreground classes
CR = C + 1   # refine head classes (background + C)
CW = C + CR  # stacked critical-path roi heads: cls | r1 = 41
PA = 48      # w_a padded cols (fp8 DoubleRow needs 16B-aligned k-stride)
PD = 48      # w_det padded cols: W(20) | -W(20) | pad(8)
PR = 32      # w_r2 padded cols


def _emit(nc, tc, aps, NS, F, n_cores):
    NB = NS // 128   # 4 roi blocks
    KT = F // 128    # 32 contraction slices
    KH = KT // 2     # roi chunk size
    KQ = KT // 4     # frm/ctx chunk size
    NP = NB * 2      # (block, supervision) pairs, index b*2+s
    group = [list(range(n_cores))]
    GW = 281  # AllGather row: vm[41] | boxes+areas[200] | z[20] | s1[20]

    (fc7, w_a, w_det, w_r2, b_a, b_r2, bxw, lab, out) = aps

    const = tc.alloc_tile_pool(name="const", bufs=1)
    st = tc.alloc_tile_pool(name="st", bufs=1)
    pst = tc.alloc_tile_pool(name="pst", bufs=2, space="PSUM")
    pss = tc.alloc_tile_pool(name="pss", bufs=2, space="PSUM")
    psa = tc.alloc_tile_pool(name="psa", bufs=1, space="PSUM")
    dp = tc.alloc_tile_pool(name="dp", bufs=1, space="DRAM")
    psc = tc.alloc_tile_pool(name="psc", bufs=1, space="PSUM")

    # ---------------- DMA issue order = arrival order ----------------------
    fc7_sb = st.tile([128, 3 * KT, NS], dt.float8e4)   # roi | frm | ctx
    w_a_sb = const.tile([128, KT, PA], dt.float8e4)
    nc.sync.dma_start(w_a_sb, w_a)
    nc.sync.dma_start(fc7_sb[:, 0:KT, :], fc7[:, 0:KT, :])
    w_det_sb = const.tile([128, KT, PD], dt.float8e4)
    nc.scalar.dma_start(w_det_sb, w_det)
    b_a_sb = const.tile([CW, 1], dt.float32)
    nc.sync.dma_start(b_a_sb, b_a[:, None])
    b_r2_sb = const.tile([CR, 1], dt.float32)
    nc.sync.dma_start(b_r2_sb, b_r2[:, None])
    bxw_sb = st.tile([128, NB, 5], dt.float32)
    nc.sync.dma_start(bxw_sb, bxw)
    labrow_i = st.tile([1, C], dt.int32)
    nc.sync.dma_start(labrow_i, lab)
    nc.scalar.dma_start(fc7_sb[:, KT:2 * KT, :], fc7[:, KT:2 * KT, :])
    nc.sync.dma_start(fc7_sb[:, 2 * KT:3 * KT, :], fc7[:, 2 * KT:3 * KT, :])
    w_r2_sb = const.tile([128, KT, PR], dt.float8e4)
    nc.scalar.dma_start(w_r2_sb, w_r2)
    boxes_nat = bxw_sb[:, :, 0:4]
    isw_col = bxw_sb[:, :, 4:5]     # [128, NB, 1]

    # ---------------- constants ----------------
    ident = const.tile([128, 128], dt.float32)
    make_identity(nc, ident)
    ones_col = const.tile([128, 1], dt.float32)
    nc.vector.memset(ones_col, 1.0)
    ones_row = const.tile([1, 128], dt.float32)
    nc.vector.memset(ones_row, 1.0)
    ones_pp = const.tile([128, 128], dt.float32)
    nc.vector.memset(ones_pp, 1.0)
    iota_i = const.tile([128, CR], dt.int32)
    nc.gpsimd.iota(iota_i, pattern=[[1, CR]], base=0, channel_multiplier=0)
    iota_f = const.tile([128, CR], dt.float32)
    nc.vector.tensor_copy(iota_f, iota_i)
    iota_m1k = const.tile([128, C], dt.float32)
    nc.vector.tensor_scalar_add(iota_m1k, iota_f[:, :C], -1000.0)
    warmln = const.tile([1, 1], dt.float32)
    nc.scalar.activation(warmln, ones_col[0:1, :], Act.Ln)   # preload Ln table

    labrow_f = st.tile([1, C], dt.float32)
    nc.vector.tensor_copy(labrow_f, labrow_i)

    # roi areas + boxes|area pack for the sel gather (early, off critical path)
    ab_all = st.tile([128, NB, 1], dt.float32)
    tw = st.tile([128, NB, 1], dt.float32)
    nc.vector.tensor_tensor(ab_all, bxw_sb[:, :, 2:3], bxw_sb[:, :, 0:1], Alu.subtract)
    nc.vector.tensor_scalar_add(ab_all, ab_all, 1.0)
    nc.vector.tensor_tensor(tw, bxw_sb[:, :, 3:4], bxw_sb[:, :, 1:2], Alu.subtract)
    nc.vector.tensor_scalar_add(tw, tw, 1.0)
    nc.vector.tensor_mul(ab_all, ab_all, tw)
    bxa = st.tile([128, NB, 5], dt.float32)
    nc.vector.tensor_copy(bxa[:, :, 0:4], boxes_nat)
    nc.vector.tensor_copy(bxa[:, :, 4:5], ab_all)

    # (1-mask)*1e30 broadcast over the 8 core-rows (filler, local-only):
    # added to negative classes' query areas so their IoU becomes ~0
    big_row = st.tile([1, 2 * C], dt.float32)
    nc.vector.tensor_scalar(big_row[:, 0:C], labrow_f, 0.0, 1e30, Alu.is_lt, Alu.mult)
    nc.vector.tensor_copy(big_row[:, C:2 * C], big_row[:, 0:C])
    ps_m = pss.tile([128, 512], dt.float32, tag="mm")
    nc.tensor.matmul(ps_m[0:n_cores, 0:2 * C], ones_row[0:1, 0:n_cores], big_row,
                     start=True, stop=True)
    bigmask8 = st.tile([n_cores, 2 * C], dt.float32)
    nc.vector.tensor_copy(bigmask8, ps_m[0:n_cores, 0:2 * C])

    # ---------------- main GEMM (bf16) -------------------------------------
    # det = frm@W + ctx@(-W): accumulate both into one PSUM bank; [W|-W] is
    # host-packed so no vector subtract is needed.
    scoresA = psc.tile([128, NS], dt.float32)   # rows 0:CW = cls | r1
    scoresB = psc.tile([128, NS], dt.float32)   # rows 0:C  = det (frm - ctx)
    DR = mybir.MatmulPerfMode.DoubleRow
    for j in range(KT // 2):
        nc.tensor.matmul(scoresA[0:CW, :], w_a_sb[:, 2 * j:2 * j + 2, 0:CW],
                         fc7_sb[:, 2 * j:2 * j + 2, :],
                         start=(j == 0), stop=(j == KT // 2 - 1), perf_mode=DR)
    for h in range(2):
        for j in range(KQ):
            kk = h * KH + 2 * j
            nc.tensor.matmul(scoresB[0:C, :], w_det_sb[:, kk:kk + 2, 0:C],
                             fc7_sb[:, KT + kk:KT + kk + 2, :],
                             start=(kk == 0), stop=False, perf_mode=DR)
        for j in range(KQ):
            kk = h * KH + 2 * j
            nc.tensor.matmul(scoresB[0:C, :], w_det_sb[:, kk:kk + 2, C:2 * C],
                             fc7_sb[:, 2 * KT + kk:2 * KT + kk + 2, :],
                             start=False, stop=(kk == KT - 2), perf_mode=DR)

    # ---------------- roi-major stats (critical path to the AllGather) -----
    CD = CW + C  # 61 packed roi-major columns
    sA = st.tile([CW, NS], dt.float32)
    nc.scalar.activation(sA, scoresA[0:CW, :], Act.Identity, bias=b_a_sb)
    dE = st.tile([C, NS], dt.float32)
    nc.scalar.activation(dE, scoresB[0:C, :], Act.Exp)

    rme = st.tile([128, NB, CD], dt.float32)     # cls|r1|exp(det), roi-major
    for b in range(NB):
        bsl = bass.ts(b, 128)
        ptx = pst.tile([128, 128], dt.float32, tag="pt")
        nc.tensor.transpose(ptx[:, 0:CW], sA[:, bsl], ident[0:CW, 0:CW])
        nc.tensor.transpose(ptx[:, CW:CD], dE[:, bsl], ident[0:C, 0:C])
        nc.vector.tensor_copy(rme[:, b, :], ptx[:, 0:CD])
    rm = rme[:, :, 0:CW]
    ed = rme[:, :, CW:CD]

    e = st.tile([128, NB, CW], dt.float32)       # exp(cls)|exp(r1)
    nc.scalar.activation(e, rm, Act.Exp)
    ec = e[:, :, 0:C]
    er = e[:, :, C:CW]
    scls = st.tile([128, NB, 1], dt.float32)
    nc.vector.reduce_sum(scls, ec, axis=AX.X)
    sr1 = st.tile([128, NB, 1], dt.float32)
    nc.vector.reduce_sum(sr1, er, axis=AX.X)
    rb1 = st.tile([128, NB, 1], dt.float32)
    nc.vector.reciprocal(rb1, scls)
    nc.vector.tensor_mul(rb1, rb1, isw_col)
    rb2 = st.tile([128, NB, 1], dt.float32)
    nc.vector.reciprocal(rb2, sr1)
    nc.vector.tensor_mul(rb2, rb2, isw_col)

    pq = st.tile([128, NB, CW], dt.float32)      # p1(20) | q2(21)
    nc.vector.tensor_mul(pq[:, :, 0:C], ec, ed)
    nc.vector.tensor_tensor(
        pq[:, :, 0:C], pq[:, :, 0:C], rb1.to_broadcast([128, NB, C]), Alu.mult)
    nc.vector.tensor_tensor(
        pq[:, :, C:CW], er, rb2.to_broadcast([128, NB, CR]), Alu.mult)

    # z / s1 partial sums: [1, 40] row via ones-matmul over rois
    zsp = st.tile([128, NB, 2 * C], dt.float32)  # exp(det) | cls*exp(det)
    nc.vector.tensor_copy(zsp[:, :, 0:C], ed)
    nc.vector.tensor_mul(zsp[:, :, C:2 * C], rm[:, :, 0:C], ed)
    ps_z = pss.tile([128, 512], dt.float32, tag="mm")
    for b in range(NB):
        nc.tensor.matmul(ps_z[0:1, 0:2 * C], ones_col, zsp[:, b, :],
                         start=(b == 0), stop=(b == NB - 1))
    zrow = st.tile([1, 2 * C], dt.float32)
    nc.vector.tensor_copy(zrow, ps_z[0:1, 0:2 * C])

    # per-class max over rois: free-dim max over blocks, then a GpSimd
    # cross-partition all-reduce (result is already broadcast to all rows)
    pmax = st.tile([128, CW], dt.float32)
    nc.vector.tensor_reduce(pmax, pq.rearrange("p b c -> p c b"),
                            axis=AX.X, op=Alu.max)
    vmP = st.tile([128, CW], dt.float32)
    nc.gpsimd.partition_all_reduce(
        vmP, pmax, channels=128, reduce_op=bass_isa.ReduceOp.max)

    sel = st.tile([128, NB, CW], dt.float32)
    nc.vector.tensor_tensor(
        sel, pq, vmP[:, None, :].to_broadcast([128, NB, CW]), Alu.is_equal)
    psq = psa.tile([128, 512], dt.float32, tag="acc")
    for b in range(NB):
        nc.tensor.matmul(psq[0:5, 0:CW], bxa[:, b, :], sel[:, b, :],
                         start=(b == 0), stop=(b == NB - 1))
    bc_sb = st.tile([5, 2 * C], dt.float32)      # winner boxes+areas, s-paired
    nc.vector.tensor_copy(bc_sb[:, 0:C], psq[0:5, 0:C])
    nc.vector.tensor_copy(bc_sb[:, C:2 * C], psq[0:5, CR:CW])   # skip bg col
    vmz = st.tile([1, 81], dt.float32)           # vm row | z | s1
    nc.vector.tensor_copy(vmz[:, 0:CW], vmP[0:1, :])
    nc.vector.tensor_copy(vmz[:, CW:81], zrow)

    # ---------------- G1: AllGather of all cross-core state ----------------
    # layout: vm[0:41] | z,s1[41:81] | boxes+areas[81:281]
    g1_in = dp.tile([GW], dt.float32)
    g1_out = dp.tile([n_cores * GW], dt.float32)
    nc.scalar.dma_start(g1_in[0:81], vmz)
    nc.sync.dma_start(g1_in[81:281], bc_sb)
    nc.gpsimd.collective_compute(
        "AllGather", Alu.bypass, replica_groups=group,
        ins=[g1_in.opt()], outs=[g1_out.opt()],
    )

    # ---- collective-latency filler: r2 GEMM + both heads' log-softmax -----
    # scoresR reuses psq's PSUM buffer (psa pool): the WAR dependency keeps
    # the PE from hoisting the r2 GEMM in front of the argmax/gather chain.
    scoresR = psa.tile([128, 512], dt.float32, tag="acc")
    for j in range(KT // 2):
        nc.tensor.matmul(scoresR[0:CR, :], w_r2_sb[:, 2 * j:2 * j + 2, 0:CR],
                         fc7_sb[:, 2 * j:2 * j + 2, :],
                         start=(j == 0), stop=(j == KT // 2 - 1), perf_mode=DR)
    r2c = st.tile([CR, NS], dt.float32)
    nc.scalar.activation(r2c, scoresR[0:CR, :], Act.Identity, bias=b_r2_sb)
    r2m = st.tile([128, NB, CR], dt.float32)
    for b in range(NB):
        bsl = bass.ts(b, 128)
        ptr = pst.tile([128, 128], dt.float32, tag="pt")
        nc.tensor.transpose(ptr[:, 0:CR], r2c[:, bsl], ident[0:CR, 0:CR])
        nc.vector.tensor_copy(r2m[:, b, :], ptr[:, 0:CR])
    er2 = st.tile([128, NB, CR], dt.float32)
    nc.scalar.activation(er2, r2m, Act.Exp)
    sr2 = st.tile([128, NB, 1], dt.float32)
    nc.vector.reduce_sum(sr2, er2, axis=AX.X)

    # log-probs for both supervisions: x - ln(sum exp x); pairs (b, s)
    xs = st.tile([128, NB, 2, CR], dt.float32)
    ln1 = st.tile([128, NB, 1], dt.float32)
    nc.scalar.activation(ln1, sr1, Act.Ln)
    nc.vector.tensor_tensor(
        xs[:, :, 0, :], rm[:, :, C:CW],
        ln1.to_broadcast([128, NB, CR]), Alu.subtract)
    ln2 = st.tile([128, NB, 1], dt.float32)
    nc.scalar.activation(ln2, sr2, Act.Ln)
    nc.vector.tensor_tensor(
        xs[:, :, 1, :], r2m, ln2.to_broadcast([128, NB, CR]), Alu.subtract)

    # ---------------- G1 readback + cross-core combine ----------------
    g_sb = st.tile([n_cores, GW], dt.float32)
    nc.sync.dma_start(g_sb, g1_out.rearrange("(r w) -> r w", r=n_cores))
    vmx = st.tile([n_cores, CW], dt.float32)
    nc.gpsimd.partition_all_reduce(
        vmx, g_sb[:, 0:CW], channels=n_cores, reduce_op=bass_isa.ReduceOp.max
    )
    selc = st.tile([n_cores, CW], dt.float32)
    nc.vector.tensor_tensor(selc, g_sb[:, 0:CW], vmx, Alu.is_equal)
    masked = st.tile([n_cores, 240], dt.float32)
    mview = masked[:, 0:200].rearrange("p (co s c) -> p co s c", co=5, s=2)
    gview = g_sb[:, 81:281].rearrange("p (co s c) -> p co s c", co=5, s=2)
    nc.vector.tensor_tensor(
        mview[:, :, 0, :], gview[:, :, 0, :],
        selc[:, None, 0:C].to_broadcast([n_cores, 5, C]), Alu.mult,
    )
    nc.vector.tensor_tensor(
        mview[:, :, 1, :], gview[:, :, 1, :],
        selc[:, None, CR:CW].to_broadcast([n_cores, 5, C]), Alu.mult,
    )
    nc.vector.tensor_tensor(masked[:, 160:200], masked[:, 160:200],
                            bigmask8, Alu.add)
    nc.vector.tensor_copy(masked[:, 200:240], g_sb[:, CW:81])
    # one matmul: ones[8,128].T @ masked -> core-sums broadcast to all rows
    ps_q = pss.tile([128, 512], dt.float32, tag="mm")
    nc.tensor.matmul(ps_q[:, 0:240], ones_pp[0:n_cores, :], masked,
                     start=True, stop=True)
    QA = st.tile([128, 240], dt.float32)   # x1|y1|x2|y2|area pairs, z|s1 tail
    nc.vector.tensor_copy(QA, ps_q[:, 0:240])

    # ---------------- batched IoU / assignment / loss ----------------------
    W2 = 2 * C
    def qb(lo):   # query coord block [128, 1, 2C] -> [128, NB, 2C]
        return QA[:, None, lo:lo + W2].to_broadcast([128, NB, W2])
    def bb(i):    # per-block box coord [128, NB, 1] -> [128, NB, 2C]
        return boxes_nat[:, :, i:i + 1].to_broadcast([128, NB, W2])

    aqab = st.tile([128, NB, W2], dt.float32)    # area_q + area_b
    nc.vector.tensor_tensor(aqab, qb(160), ab_all.to_broadcast([128, NB, W2]), Alu.add)
    xi1 = st.tile([128, NB, W2], dt.float32)
    nc.vector.tensor_tensor(xi1, qb(0), bb(0), Alu.max)
    yi1 = st.tile([128, NB, W2], dt.float32)
    nc.vector.tensor_tensor(yi1, qb(40), bb(1), Alu.max)
    xi2 = st.tile([128, NB, W2], dt.float32)
    nc.vector.tensor_tensor(xi2, qb(80), bb(2), Alu.min)
    yi2 = st.tile([128, NB, W2], dt.float32)
    nc.vector.tensor_tensor(yi2, qb(120), bb(3), Alu.min)
    nc.vector.tensor_tensor(xi2, xi2, xi1, Alu.subtract)
    nc.vector.tensor_scalar(xi2, xi2, 1.0, 0.0, Alu.add, Alu.max)   # iw
    nc.vector.tensor_tensor(yi2, yi2, yi1, Alu.subtract)
    nc.vector.tensor_scalar(yi2, yi2, 1.0, 0.0, Alu.add, Alu.max)   # ih
    inter = st.tile([128, NB, W2], dt.float32)
    nc.vector.tensor_mul(inter, xi2, yi2)
    un = st.tile([128, NB, W2], dt.float32)
    nc.vector.tensor_tensor(un, aqab, inter, Alu.subtract)
    nc.vector.reciprocal(un, un)
    ov = st.tile([128, NB, W2], dt.float32)
    nc.vector.tensor_mul(ov, inter, un)

    ovp = ov.rearrange("p b (s c) -> p (b s) c", s=2)   # [128, NP, C]
    mo = st.tile([128, NP, 1], dt.float32)
    nc.vector.reduce_max(mo, ovp, axis=AX.X)
    meq = st.tile([128, NP, C], dt.float32)
    nc.vector.tensor_tensor(
        meq, ovp, mo.to_broadcast([128, NP, C]), Alu.is_equal)
    nc.vector.tensor_tensor(
        meq, meq, iota_m1k[:, None, :].to_broadcast([128, NP, C]), Alu.mult)
    gt = st.tile([128, NP, 1], dt.float32)
    nc.vector.tensor_reduce(gt, meq, axis=AX.X, op=Alu.min)
    nc.vector.tensor_scalar_add(gt, gt, 1001.0)          # argmax + 1

    fg = st.tile([128, NP, 1], dt.float32)
    nc.vector.tensor_scalar(fg, mo, 0.5, None, Alu.is_gt)
    keep = st.tile([128, NP, 1], dt.float32)
    nc.vector.tensor_scalar(keep, mo, 0.1, None, Alu.is_ge)
    col = st.tile([128, NP, 1], dt.float32)
    nc.vector.tensor_mul(col, gt, fg)                    # fg ? argmax+1 : 0
    oh = st.tile([128, NP, CR], dt.float32)
    nc.vector.tensor_tensor(
        oh, iota_f[:, None, :].to_broadcast([128, NP, CR]),
        col.to_broadcast([128, NP, CR]), Alu.is_equal)
    nc.vector.tensor_mul(oh, oh, xs.rearrange("p b s c -> p (b s) c"))
    lpsel = st.tile([128, NP, 1], dt.float32)
    nc.vector.reduce_sum(lpsel, oh, axis=AX.X)

    stats = st.tile([128, 16], dt.float32)               # wl[8] | keep[8]
    wv = stats[:, 0:NP].rearrange("p (b s) -> p b s", s=2)
    kv = keep.rearrange("p (b s) o -> p b (s o)", s=2)   # [128, NB, 2]
    nc.vector.tensor_tensor(wv, kv, isw_col.to_broadcast([128, NB, 2]), Alu.mult)
    nc.vector.tensor_mul(
        stats[:, 0:NP], stats[:, 0:NP],
        lpsel.rearrange("p n o -> p (n o)"))
    nc.vector.tensor_copy(stats[:, NP:2 * NP], keep.rearrange("p n o -> p (n o)"))
    ps_l = psa.tile([128, 512], dt.float32, tag="acc")
    nc.tensor.matmul(ps_l[0:16, 0:1], stats, ones_col, start=True, stop=True)
    lsum = st.tile([16, 1], dt.float32)
    nc.vector.tensor_copy(lsum, ps_l[0:16, 0:1])

    # hinge is finished on host from the global z|s1 sums (identical on
    # every core after the AllGather)
    nc.scalar.dma_start(out[16:56], QA[0:1, 200:240])
    nc.sync.dma_start(out[0:16], lsum[:, 0])

    for pool in (psc, dp, psa, pss, pst, st, const):
        pool.release()


def build_program(NS=512, F=4096, n_cores=8):
    nc = bacc.Bacc(
        "TRN2", target_bir_lowering=False, debug=False, num_devices=n_cores
    )
    KT = F // 128
    fc7 = nc.dram_tensor("fc7", [128, 3 * KT, NS], dt.float8e4, kind="ExternalInput").ap()
    w_a = nc.dram_tensor("w_a", [128, KT, PA], dt.float8e4, kind="ExternalInput").ap()
    w_det = nc.dram_tensor("w_det", [128, KT, PD], dt.float8e4, kind="ExternalInput").ap()
    w_r2 = nc.dram_tensor("w_r2", [128, KT, PR], dt.float8e4, kind="ExternalInput").ap()
    b_a = nc.dram_tensor("b_a", [CW], dt.float32, kind="ExternalInput").ap()
    b_r2 = nc.dram_tensor("b_r2", [CR], dt.float32, kind="ExternalInput").ap()
    bxw = nc.dram_tensor("bxw", [128, NS // 128, 5], dt.float32, kind="ExternalInput").ap()
    lab = nc.dram_tensor("lab", [1, C], dt.int32, kind="ExternalInput").ap()
    out = nc.dram_tensor("out", [56], dt.float32, kind="ExternalOutput").ap()
    aps = (fc7, w_a, w_det, w_r2, b_a, b_r2, bxw, lab, out)
    with tile.TileContext(nc) as tc:
        _emit(nc, tc, aps, NS, F, n_cores)
    nc.compile()
    return nc


def _pack_fc7(a_t_bf16, sl, F):
    # [F, NS] bf16 slice -> [128, KT, NS] with contiguous per-partition runs
    return np.ascontiguousarray(
        a_t_bf16[:, sl].reshape(F // 128, 128, -1).transpose(1, 0, 2))


def _pack_w(w, pad_cols):
    F, cols = w.shape
    wp = np.zeros((F, pad_cols), np.float32)
    wp[:, 0:cols] = w
    return np.ascontiguousarray(
        wp.astype(ml_dtypes.float8_e4m3fn).reshape(F // 128, 128, pad_cols)
        .transpose(1, 0, 2))


def make_in_maps(inputs, NS, n_cores):
    f32 = np.float32
    bf16 = ml_dtypes.bfloat16
    fp8 = ml_dtypes.float8_e4m3fn
    w_a = _pack_w(np.concatenate(
        [np.asarray(inputs["W_cls"], f32), np.asarray(inputs["W_r1"], f32)], axis=1), PA)
    wd = np.asarray(inputs["W_det"], f32)
    w_det = _pack_w(np.concatenate([wd, -wd], axis=1), PD)
    w_r2 = _pack_w(np.asarray(inputs["W_r2"], f32), PR)
    b_a = np.ascontiguousarray(np.concatenate(
        [np.asarray(inputs["b_cls"]), np.asarray(inputs["b_r1"])]), f32)
    b_r2 = np.ascontiguousarray(np.asarray(inputs["b_r2"]), f32)
    boxes = np.asarray(inputs["ss_boxes"], f32)[:, 1:5]
    iswf = np.asarray(inputs["IS_weight"], f32)[:, 0]
    lab = np.ascontiguousarray(np.asarray(inputs["image_level_label"]), np.int32)
    roi = np.asarray(inputs["fc7_roi"], f32).T.astype(fp8)
    frm = np.asarray(inputs["fc7_frame"], f32).T.astype(fp8)
    ctxm = np.asarray(inputs["fc7_context"], f32).T.astype(fp8)
    F = roi.shape[0]
    NB = NS // 128

    in_maps = []
    for c in range(n_cores):
        sl = slice(c * NS, (c + 1) * NS)
        bsh = boxes[sl].reshape(NB, 128, 4).transpose(1, 0, 2)
        ish = iswf[sl].reshape(NB, 128).T[:, :, None]
        bxw = np.ascontiguousarray(np.concatenate([bsh, ish], axis=2), f32)
        in_maps.append({
            "fc7": np.ascontiguousarray(np.concatenate(
                [_pack_fc7(roi, sl, F), _pack_fc7(frm, sl, F),
                 _pack_fc7(ctxm, sl, F)], axis=1)),
            "w_a": w_a, "w_det": w_det, "w_r2": w_r2,
            "b_a": b_a, "b_r2": b_r2,
            "bxw": bxw, "lab": lab,
        })
    return in_maps


_PROG_CACHE = {}


def _get_prog(NS, F, n_cores):
    key = (NS, F, n_cores)
    if key not in _PROG_CACHE:
        _PROG_CACHE[key] = build_program(NS, F, n_cores)
    return _PROG_CACHE[key]


def finish(results, lab, n_cores=8):
    # host-side gather/unshard: combine the per-core partial sums
    parts = np.stack([np.asarray(results[i]["out"], np.float64).reshape(56)
                      for i in range(n_cores)])
    wl = parts[:, 0:8].sum(axis=0)      # per (b, s=idx%2) weighted log-probs
    kp = parts[:, 8:16].sum(axis=0)     # per (b, s) keep counts
    rl1 = -wl[0::2].sum() / kp[0::2].sum()
    rl2 = -wl[1::2].sum() / kp[1::2].sum()
    z = parts[0, 16:36]
    s1 = parts[0, 36:56]
    h = np.maximum(0.0, 1.0 - lab * (s1 / z)).sum()
    return np.float32(h / C + 0.1 * rl1 + 0.1 * rl2)


def kernel(**inputs):
    n_cores = 8
    N, F = inputs["fc7_roi"].shape
    NS = N // n_cores
    prog = _get_prog(NS, F, n_cores)
    in_maps = make_in_maps(inputs, NS, n_cores)
    res = run_bass_kernel_spmd(prog, in_maps, list(range(n_cores))).results
    lab = np.asarray(inputs["image_level_label"], np.float64)[0]
    return finish(res, lab, n_cores)
